# revision 26
# baseline (speedup 1.0000x reference)
"""Trainium2 Bass kernel for nn_MultiHeadAttention (conv-projected MHA).

Reference (B=4, C=512, L=2048, H=8, D=64):
    qc = conv1d_same(q, wq)            # [B, C, L]
    qh = qc.reshape(B, -1, H, D).transpose(0,2,1,3)
    ... attn = softmax(qh @ kh / D); out = attn @ vh
    out -> [B, C, L] -> conv1d_same(out, fc)

LAYOUT FACT: the row-major reshape of [C, L=2048] to [n=2048, H=8, D=64]
means attention position n = (c, j=l//512) and head/feature = l%512.
Heads slice along L; the 2048 positions are (channel, quarter) pairs.

Sharding: 8 cores = (batch, L-half).  A core owns output columns
l' in [half*1024, half*1024+1024), i.e. QUERY groups j in {2half, 2half+1}
for all heads.  Convs are computed on the own half only; the pair of
cores of a batch exchanges k/v conv results.

RANK-UNIFORM EXCHANGE: instead of AllGather (whose output is rank-
indexed), each pair runs AllReduce(add) and every core recovers the
peer half as  peer = sum - own  (gpsimd subtract).  All attention
indexing is LOCAL: key chunks 0-7 = own j-groups, 8-15 = peer's.
Softmax order over keys is irrelevant, so this is exact.

TWO-PHASE SCHEDULE (the scalar-engine exp chain, ~170us, is the
critical resource; it starts ~25us in and never stops):
  phase 1: per head-pair, mm1+exp over OWN key chunks (available right
    after the local k conv), partial O accumulated and parked in SBUF.
    PE fillers pumped between exp rounds: v conv, q conv, v AllReduce.
  phase 2: mm1+exp over PEER chunks (landed via AllReduce during
    phase 1), mm2 peer accumulation + combine with parked partial,
    transpose + 1/denominator normalize, fc conv pumped as fillers.

Boundary halo columns (l' = qlo-1 / qhi+1) belong to peer query groups;
they are exchanged post-attention with a tiny AllReduce (sum - own,
masked by host scalars) instead of being recomputed.
"""

import os
from collections import deque

import numpy as np
import ml_dtypes

B, C, L = 4, 512, 2048
H, D = 8, 64
NCORES = 8
HALF = L // 2
QW = HALF + 2            # attn_out buffer cols: halo + 1024 + halo
KC = 16                  # key chunks (local): 0-7 own, 8-15 peer
CIN_CH = 4
COUT_CH = 4
VSLOT = D + 1            # 65: V columns + ones column
V16 = 16 * VSLOT

BF16 = ml_dtypes.bfloat16

_CACHE = {}
_LAST_IN_MAPS = None
_LAST_RESULTS = None


def _build(flags):
    use_qb, use_kb, use_vb, use_fb = flags
    import concourse.bass as bass
    import concourse.bacc as bacc
    import concourse.tile as tile
    from concourse import mybir
    from concourse.masks import make_identity
    from contextlib import ExitStack

    f32 = mybir.dt.float32
    bf16 = mybir.dt.bfloat16
    PAIRS = [[0, 1], [2, 3], [4, 5], [6, 7]]

    def bcast_rows(ap, nrows):
        return bass.AP(tensor=ap.tensor, offset=ap.offset,
                       ap=[[0, nrows]] + [list(d) for d in ap.ap[1:]])

    nc = bacc.Bacc("TRN2", target_bir_lowering=False, debug=False,
                   num_devices=NCORES)

    q_in_d = nc.dram_tensor("q_in", [C, 1026], bf16, kind="ExternalInput").ap()
    k_in_d = nc.dram_tensor("k_in", [C, 1026], bf16, kind="ExternalInput").ap()
    v_in_d = nc.dram_tensor("v_in", [C, 1026], bf16, kind="ExternalInput").ap()
    k_srcA = nc.dram_tensor("k_srcA", [128, 4, C], bf16).ap()
    k_sumA = nc.dram_tensor("k_sumA", [128, 4, C], bf16).ap()
    k_srcB = nc.dram_tensor("k_srcB", [128, 4, C], bf16).ap()
    k_sumB = nc.dram_tensor("k_sumB", [128, 4, C], bf16).ap()
    v_src = nc.dram_tensor("v_src", [128, CIN_CH, V16], bf16).ap()
    v_sumd = nc.dram_tensor("v_sum", [128, CIN_CH, V16], bf16).ap()
    h_src = nc.dram_tensor("h_src", [128, CIN_CH, 2], bf16).ap()
    h_sumd = nc.dram_tensor("h_sum", [128, CIN_CH, 2], bf16).ap()
    wq_d = nc.dram_tensor("wq", [3, C, C], bf16, kind="ExternalInput").ap()
    wk_d = nc.dram_tensor("wk", [3, C, C], bf16, kind="ExternalInput").ap()
    wv_d = nc.dram_tensor("wv", [3, C, C], bf16, kind="ExternalInput").ap()
    wfc_d = nc.dram_tensor("wfc", [3, C, C], bf16, kind="ExternalInput").ap()
    # hm[0]=mL gates left halo (buffer col 0), hm[1]=mR gates right
    # halo (buffer col 1025)
    hm_d = nc.dram_tensor("hm", [1, 2], f32, kind="ExternalInput").ap()
    qb_d = kb_d = vb_d = fb_d = None
    if use_qb:
        qb_d = nc.dram_tensor("qb", [1, C], f32, kind="ExternalInput").ap()
    if use_kb:
        kb_d = nc.dram_tensor("kb", [1, C], f32, kind="ExternalInput").ap()
    if use_vb:
        vb_d = nc.dram_tensor("vb", [128, CIN_CH], f32, kind="ExternalInput").ap()
    if use_fb:
        fb_d = nc.dram_tensor("fb", [128, CIN_CH], f32, kind="ExternalInput").ap()
    out_d = nc.dram_tensor("out", [C, HALF], f32, kind="ExternalOutput").ap()

    dbg = bool(os.environ.get("BASS_DEBUG_DUMP"))
    if dbg:
        dbg_kt = nc.dram_tensor("dbg_kt", [128, KC, C], bf16,
                                kind="ExternalOutput").ap()
        dbg_qt = nc.dram_tensor("dbg_qt", [128, 8, C], bf16,
                                kind="ExternalOutput").ap()
        dbg_vs = nc.dram_tensor("dbg_vs", [128, CIN_CH, 32 * VSLOT], bf16,
                                kind="ExternalOutput").ap()
        dbg_ao = nc.dram_tensor("dbg_ao", [128, CIN_CH, QW], bf16,
                                kind="ExternalOutput").ap()
        dbg_e = nc.dram_tensor("dbg_exp", [128, 8, 512], bf16,
                               kind="ExternalOutput").ap()

    with tile.TileContext(nc) as tc, ExitStack() as ctx:
        consts = ctx.enter_context(tc.tile_pool(name="consts", bufs=1))
        # PSUM (8 banks): shared (convs/fc/transposes) 2 + scores 2x2 + o 2
        shared_ps = ctx.enter_context(
            tc.tile_pool(name="shared_ps", bufs=2, space="PSUM"))
        scores_ps = ctx.enter_context(
            tc.tile_pool(name="scores_ps", bufs=2, space="PSUM"))
        o_ps = ctx.enter_context(tc.tile_pool(name="o_ps", bufs=2, space="PSUM"))
        exp_pool = ctx.enter_context(tc.tile_pool(name="exp_pool", bufs=4))
        o_sb_pool = ctx.enter_context(tc.tile_pool(name="o_sb_pool", bufs=3))
        small = ctx.enter_context(tc.tile_pool(name="small", bufs=4))
        fc_pool = ctx.enter_context(tc.tile_pool(name="fc_pool", bufs=2))
        sum_pool = ctx.enter_context(tc.tile_pool(name="sum_pool", bufs=1))
        halo_pool = ctx.enter_context(tc.tile_pool(name="halo_pool", bufs=3))
        tmp_ctx = ExitStack()
        tmp_pool = tmp_ctx.enter_context(tc.tile_pool(name="tmp_pool", bufs=1))

        # ---- constants / inputs (split DMAs, just-in-time order) ----
        wq_sb = consts.tile([128, 3, CIN_CH, C], bf16)
        wk_sb = consts.tile([128, 3, CIN_CH, C], bf16)
        wv_sb = consts.tile([128, 3, CIN_CH, C], bf16)
        wfc_sb = consts.tile([128, 3, CIN_CH, C], bf16)
        q_in = tmp_pool.tile([128, CIN_CH, 1026], bf16)
        k_in = tmp_pool.tile([128, CIN_CH, 1026], bf16)
        v_in = tmp_pool.tile([128, CIN_CH, 1026], bf16)

        def dma_w(sb, d):  # per-tap pieces so the first matmul starts early
            for t in range(3):
                nc.sync.dma_start(
                    out=sb[:, t], in_=d[t].rearrange("(ki p) co -> p ki co",
                                                     p=128))

        def dma_x(sb, d):  # per-cin-chunk pieces
            r = d.rearrange("(ki p) l -> ki p l", p=128)
            for ki in range(CIN_CH):
                nc.sync.dma_start(out=sb[:, ki], in_=r[ki])

        # wk tap0 + k_in first: the first conv unit needs exactly these
        nc.sync.dma_start(out=wk_sb[:, 0],
                          in_=wk_d[0].rearrange("(ki p) co -> p ki co", p=128))
        dma_x(k_in, k_in_d)
        for t in (1, 2):
            nc.sync.dma_start(out=wk_sb[:, t],
                              in_=wk_d[t].rearrange("(ki p) co -> p ki co",
                                                    p=128))
        dma_w(wq_sb, wq_d)
        dma_x(q_in, q_in_d)
        dma_w(wv_sb, wv_d)
        dma_x(v_in, v_in_d)
        dma_w(wfc_sb, wfc_d)

        hm_sb = consts.tile([128, 2], f32)
        nc.sync.dma_start(out=hm_sb, in_=bcast_rows(hm_d, 128))
        ident = consts.tile([128, 128], bf16)
        make_identity(nc, ident)

        qb_bc = kb_bc = vb_sb = fb_sb = None
        if use_qb:
            qb_bc = consts.tile([128, C], f32)
            nc.sync.dma_start(out=qb_bc, in_=bcast_rows(qb_d, 128))
        if use_kb:
            kb_bc = consts.tile([128, C], f32)
            nc.sync.dma_start(out=kb_bc, in_=bcast_rows(kb_d, 128))
        if use_vb:
            vb_sb = consts.tile([128, CIN_CH], f32)
            nc.sync.dma_start(out=vb_sb, in_=vb_d)
        if use_fb:
            fb_sb = consts.tile([128, CIN_CH], f32)
            nc.sync.dma_start(out=fb_sb, in_=fb_d)

        # kT slot = jp*4 + h//2 (jp = local key j-group); 0-7 own conv,
        # 8-15 peer via AllReduce-minus-own.
        kT = consts.tile([128, KC, C], bf16)
        qT = consts.tile([128, 8, C], bf16)
        # v slot (jp*8+h), stride VSLOT; 0-15 own, 16-31 peer; pad for
        # the 128-wide mm2 lhsT overread
        v_slots = consts.tile([128, CIN_CH, 32 * VSLOT + 64], bf16)
        v_loc = v_slots[:, :, 0:V16]
        v_peer = v_slots[:, :, V16:2 * V16]
        attn_out = consts.tile([128, CIN_CH, QW], bf16)
        # parked partial O (own-key half) per (h, jj): slot jj*8+h
        o_part = consts.tile([VSLOT, 16, 512], bf16)
        nc.vector.memset(v_loc, 1.0)             # ones cols; data overwritten
        nc.vector.memset(v_slots[:, :, 32 * VSLOT:], 0.0)  # lhsT overread pad

        def cc(src, dst):
            nc.gpsimd.collective_compute(
                kind="AllReduce", op=mybir.AluOpType.add,
                replica_groups=PAIRS, ins=[src], outs=[dst])

        # ---- conv building blocks ----
        def conv_units(x_in, w_sb, bias_bc, out_sb, slot, col0):
            """Transposed conv tile split into 4 pump units."""
            ps = shared_ps.tile([128, 512], f32, name="convps")
            units = []
            for u in range(3):
                def mmu(u=u, ps=ps):
                    for n in range(u * 4, u * 4 + 4):
                        t, ki = n // CIN_CH, n % CIN_CH
                        nc.tensor.matmul(
                            ps,
                            lhsT=x_in[:, ki, col0 + t: col0 + t + 128],
                            rhs=w_sb[:, t, ki, :],
                            start=(n == 0), stop=(n == 11))
                units.append(mmu)

            def evac(ps=ps):
                dst = out_sb[:, slot, :]
                if bias_bc is not None:
                    nc.vector.tensor_add(dst, ps, bias_bc)
                else:
                    nc.vector.tensor_copy(dst, ps)
            units.append(evac)
            return units

        def v_tile_units(co, lt):
            """Normal-orientation v conv tile -> slotted layout."""
            ps = shared_ps.tile([128, 512], f32, name="convps")
            units = []
            for u in range(3):
                def mmu(u=u, ps=ps):
                    for n in range(u * 4, u * 4 + 4):
                        t, ki = n // CIN_CH, n % CIN_CH
                        nc.tensor.matmul(
                            ps,
                            lhsT=wv_sb[:, t, ki, co * 128:(co + 1) * 128],
                            rhs=v_in[:, ki, lt * 512 + t: lt * 512 + t + 512],
                            start=(n == 0), stop=(n == 11))
                units.append(mmu)

            def evac(ps=ps):
                dst = v_loc[:, co, lt * 8 * VSLOT:(lt + 1) * 8 * VSLOT] \
                    .rearrange("p (h e) -> p h e", e=VSLOT)[:, :, 0:D]
                src = ps.rearrange("p (h d) -> p h d", d=D)
                if use_vb:
                    nc.vector.tensor_scalar_add(dst, src, vb_sb[:, co:co + 1])
                else:
                    nc.vector.tensor_copy(dst, src)
            units.append(evac)
            return units

        def run_units(units):
            for u in units:
                u()

        # ---- k conv + AllReduce exchange (own half stays in slots 0-7) --
        for s in range(4):
            run_units(conv_units(k_in, wk_sb, kb_bc if use_kb else None,
                                 kT, s, s * 128))
        nc.sync.dma_start(out=k_srcA, in_=kT[:, 0:4, :])
        cc(k_srcA, k_sumA)
        for s in range(4, 8):
            run_units(conv_units(k_in, wk_sb, kb_bc if use_kb else None,
                                 kT, s, s * 128))
        nc.sync.dma_start(out=k_srcB, in_=kT[:, 4:8, :])
        cc(k_srcB, k_sumB)

        # ---- v conv BEFORE attention: the kernel is PE-bound, so this
        # prefix is not waste; pushing it into phase-1 fillers measured
        # slower (oversubscribed the per-pair act windows) ----
        for lt in range(2):
            for co in range(COUT_CH):
                run_units(v_tile_units(co, lt))
        nc.sync.dma_start(out=v_src, in_=v_loc)
        cc(v_src, v_sumd)

        # ---- first q slot, then attention starts ----
        def q_slot_units(s):
            return conv_units(q_in, wq_sb, qb_bc if use_qb else None,
                              qT, s, s * 128)

        run_units(q_slot_units(0))

        # ---- filler pump ----
        fillers = deque()

        def pump(n):
            for _ in range(n):
                if fillers:
                    fillers.popleft()()

        # q slots just-in-time (slot s consumed by pair #s)
        for s in range(1, 8):
            fillers.extend(q_slot_units(s))

        # ---- attention ----
        def mm1_round(h, m, jj, rnd, phase, exp_t):
            """One head, chunks (2rnd, 2rnd+1) of the given phase half."""
            p0 = (h % 2) * 64
            sc = scores_ps.tile([128, 2, 512], f32, name="sc")
            for jx in range(2):
                pos = rnd * 2 + jx
                jp = 2 * phase + pos // 4
                ck = pos % 4
                nc.tensor.matmul(
                    sc[:, jx, :],
                    lhsT=kT[p0:p0 + 64, jp * 4 + m,
                            ck * 128:(ck + 1) * 128],
                    rhs=qT[p0:p0 + 64, jj * 4 + m, :],
                    start=True, stop=True)
            nc.scalar.activation(
                out=exp_t[:, rnd * 2:(rnd + 1) * 2, :], in_=sc,
                func=mybir.ActivationFunctionType.Exp, scale=1.0 / D)

        def mm2_half(h, phase, exp_t):
            o = o_ps.tile([128, 512], f32, name="o")
            for pos in range(8):
                jp = 2 * phase + pos // 4
                ck = pos % 4
                base = (jp * 8 + h) * VSLOT
                nc.tensor.matmul(o, lhsT=v_slots[:, ck, base:base + 128],
                                 rhs=exp_t[:, pos, :],
                                 start=(pos == 0), stop=(pos == 7))
            return o

        def pair_rounds(m, jj, phase, pump_n=2):
            hA, hB = 2 * m, 2 * m + 1
            eA = exp_pool.tile([128, 8, 512], bf16, name="exp_t")
            eB = exp_pool.tile([128, 8, 512], bf16, name="exp_t")
            for rnd in range(4):
                mm1_round(hA, m, jj, rnd, phase, eA)
                pump(pump_n)
                mm1_round(hB, m, jj, rnd, phase, eB)
                pump(pump_n)
            return eA, eB

        def park_units(h, jj, e):
            """Deferred mm2 over own keys -> o_part, as 3 pump units."""
            st = {}

            def u1():
                st["o"] = o_ps.tile([128, 512], f32, name="o")
                for pos in range(4):
                    jp, ck = pos // 4, pos % 4
                    base = (jp * 8 + h) * VSLOT
                    nc.tensor.matmul(st["o"],
                                     lhsT=v_slots[:, ck, base:base + 128],
                                     rhs=e[:, pos, :],
                                     start=(pos == 0), stop=False)

            def u2():
                for pos in range(4, 8):
                    jp, ck = pos // 4, pos % 4
                    base = (jp * 8 + h) * VSLOT
                    nc.tensor.matmul(st["o"],
                                     lhsT=v_slots[:, ck, base:base + 128],
                                     rhs=e[:, pos, :],
                                     start=False, stop=(pos == 7))

            def u3():
                nc.vector.tensor_copy(o_part[:, jj * 8 + h, :],
                                      st["o"][0:VSLOT, :])
            return [u1, u2, u3]

        def finish(h, jj, e):
            o = mm2_half(h, 1, e)
            o_sb = o_sb_pool.tile([VSLOT, 512], bf16, name="o_sb")
            nc.vector.scalar_tensor_tensor(
                out=o_sb, in0=o_part[:, jj * 8 + h, :], scalar=1.0,
                in1=o[0:VSLOT, :],
                op0=mybir.AluOpType.mult, op1=mybir.AluOpType.add)
            lo = 1 + jj * 512 + h * D
            for ck in range(4):
                # NOT shared_ps: a pumped fc tile's accumulation may be
                # open there, and bank reuse against its later-emitted
                # evac would deadlock the PE/DVE queues
                tp = o_ps.tile([128, VSLOT], bf16, name="tp", tag="o")
                nc.tensor.transpose(tp,
                                    o_sb[:, ck * 128:(ck + 1) * 128],
                                    ident[0:VSLOT, 0:VSLOT])
                rc = small.tile([128, 1], f32, name="rc")
                nc.vector.reciprocal(rc, tp[:, D:D + 1])
                nc.vector.tensor_scalar_mul(
                    attn_out[:, ck, lo:lo + D], tp[:, 0:D], rc)
            return o_sb

        def park(h, jj, e):
            for u in park_units(h, jj, e):
                u()

        # ---- phase 1: own keys, park partial O ----
        P1 = [(m, jj) for jj in (0, 1) for m in range(4)]
        for m, jj in P1:
            # pump_n=1: spreads the q-slot fillers over ~3.5 pairs instead
            # of front-loading pairs 0-1 (which oversubscribed their act
            # windows by ~8us); slot s still lands >=2 pairs before pair s
            eA, eB = pair_rounds(m, jj, 0, pump_n=1)
            park(2 * m, jj, eA)
            pump(1)
            park(2 * m + 1, jj, eB)
            pump(1)
            if dbg and m == 0 and jj == 0:
                nc.sync.dma_start(out=dbg_e, in_=eA)

        # ---- peer halves (sum - own), deferred to here: the collectives
        # have landed by now, so the in-order DVE/DMA queues don't block
        # phase-1 work behind them ----
        ksA = sum_pool.tile([128, 4, C], bf16, name="ksum")
        nc.sync.dma_start(out=ksA, in_=k_sumA)
        nc.vector.scalar_tensor_tensor(
            out=kT[:, 8:12, :], in0=ksA, scalar=1.0, in1=kT[:, 0:4, :],
            op0=mybir.AluOpType.mult, op1=mybir.AluOpType.subtract)
        ksB = sum_pool.tile([128, 4, C], bf16, name="ksum")
        nc.sync.dma_start(out=ksB, in_=k_sumB)
        nc.vector.scalar_tensor_tensor(
            out=kT[:, 12:16, :], in0=ksB, scalar=1.0, in1=kT[:, 4:8, :],
            op0=mybir.AluOpType.mult, op1=mybir.AluOpType.subtract)
        vsum = sum_pool.tile([128, CIN_CH, V16], bf16, name="vsum")
        nc.sync.dma_start(out=vsum, in_=v_sumd)
        nc.vector.scalar_tensor_tensor(
            out=v_peer, in0=vsum, scalar=1.0, in1=v_loc,
            op0=mybir.AluOpType.mult, op1=mybir.AluOpType.subtract)

        # ---- fc conv (fillers for phase 2) ----
        def fc_units(co, lo, w):
            ps = shared_ps.tile([128, 512], f32, name="convps")
            units = []
            for u in range(3):
                def mmu(u=u, ps=ps):
                    for n in range(u * 4, u * 4 + 4):
                        t, ki = n // CIN_CH, n % CIN_CH
                        nc.tensor.matmul(
                            ps[:, 0:w],
                            lhsT=wfc_sb[:, t, ki, co * 128:(co + 1) * 128],
                            rhs=attn_out[:, ki, lo + t: lo + t + w],
                            start=(n == 0), stop=(n == 11))
                units.append(mmu)

            def evac(ps=ps):
                fc_sb = fc_pool.tile([128, 512], f32, name="fc_sb")
                if use_fb:
                    nc.vector.tensor_scalar_add(fc_sb[:, 0:w], ps[:, 0:w],
                                                fb_sb[:, co:co + 1])
                else:
                    nc.vector.tensor_copy(fc_sb[:, 0:w], ps[:, 0:w])
                nc.sync.dma_start(
                    out=out_d[co * 128:(co + 1) * 128, lo:lo + w],
                    in_=fc_sb[:, 0:w])
            units.append(evac)
            return units

        halo_sb = {}

        def halo_send():
            hsrc = halo_pool.tile([128, CIN_CH, 2], bf16, name="hsrc")
            # slot 0: my (jj0,h0,d0) col; slot 1: my (jj1,h7,d63) col
            nc.vector.tensor_copy(hsrc[:, :, 0:1], attn_out[:, :, 1:2])
            nc.vector.tensor_copy(hsrc[:, :, 1:2],
                                  attn_out[:, :, 1024:1025])
            nc.sync.dma_start(out=h_src, in_=hsrc)
            cc(h_src, h_sumd)
            halo_sb["hsrc"] = hsrc

        def halo_recv():
            # deferred: by now the halo cc has landed, so the readback
            # doesn't block later fc-output DMAs on the sync queue
            hsum = halo_pool.tile([128, CIN_CH, 2], bf16, name="hsum")
            nc.sync.dma_start(out=hsum, in_=h_sumd)
            hdiff = halo_pool.tile([128, CIN_CH, 2], f32, name="hdiff")
            nc.vector.scalar_tensor_tensor(
                out=hdiff, in0=hsum, scalar=1.0, in1=halo_sb["hsrc"],
                op0=mybir.AluOpType.mult, op1=mybir.AluOpType.subtract)
            # left halo (col 0) = peer (jj1,h7,d63); right (col 1025) =
            # peer (jj0,h0,d0); each gated by a host mask scalar
            nc.vector.tensor_scalar_mul(
                attn_out[:, :, 0:1], hdiff[:, :, 1:2], hm_sb[:, 0:1])
            nc.vector.tensor_scalar_mul(
                attn_out[:, :, 1025:1026], hdiff[:, :, 0:1], hm_sb[:, 1:2])

        # ---- phase 2: peer keys, combine, normalize, fc fillers ----
        # (0,0) and (3,1) first: their outputs feed the halo exchange.
        # The last pair gates only a narrow 128-col fc piece (small tail).
        P2 = [(0, 0), (3, 1), (1, 0), (2, 0), (3, 0), (2, 1), (0, 1), (1, 1)]
        done = set()
        for idx, (m, jj) in enumerate(P2):
            eA, eB = pair_rounds(m, jj, 1)
            finish(2 * m, jj, eA)
            pump(1)
            finish(2 * m + 1, jj, eB)
            pump(1)
            done.add((m, jj))
            if idx == 1:
                halo_send()
            if done >= {(0, 0), (1, 0), (2, 0), (3, 0)} and \
                    "fc0" not in done:
                done.add("fc0")
                halo_recv()
                for co in range(COUT_CH):
                    fillers.extend(fc_units(co, 0, 511))
            # output col i reads attn_out buffers i..i+2; jj1 head hX
            # occupies buffers 513+64X..576+64X, halo col is 1025
            if done >= {(2, 1), (3, 1)} and "fcA" not in done and \
                    "fc0" in done:
                done.add("fcA")
                for co in range(COUT_CH):
                    fillers.extend(fc_units(co, 769, 255))
            if done >= {(0, 1)} and "fcB" not in done and "fcA" in done:
                done.add("fcB")
                for co in range(COUT_CH):
                    fillers.extend(fc_units(co, 511, 128))
            if done >= {(1, 1)} and "fcC" not in done and "fcB" in done:
                done.add("fcC")
                for co in range(COUT_CH):
                    fillers.extend(fc_units(co, 639, 130))

        tmp_ctx.close()
        pump(len(fillers))

        if dbg:
            for sb, dd in ((kT, dbg_kt), (qT, dbg_qt),
                           (v_slots[:, :, 0:32 * VSLOT], dbg_vs),
                           (attn_out, dbg_ao)):
                nc.sync.dma_start(out=dd, in_=sb)

    nc.compile()
    return nc


def kernel(q, k, v, wq_w, wq_b, wk_w, wk_b, wv_w, wv_b, fc_w, fc_b):
    q = np.asarray(q, np.float32)
    k = np.asarray(k, np.float32)
    v = np.asarray(v, np.float32)
    wq_w = np.asarray(wq_w, np.float32)
    wk_w = np.asarray(wk_w, np.float32)
    wv_w = np.asarray(wv_w, np.float32)
    fc_w = np.asarray(fc_w, np.float32)
    wq_b = np.asarray(wq_b, np.float32)
    wk_b = np.asarray(wk_b, np.float32)
    wv_b = np.asarray(wv_b, np.float32)
    fc_b = np.asarray(fc_b, np.float32)

    flags = (bool(wq_b.any()), bool(wk_b.any()),
             bool(wv_b.any()), bool(fc_b.any()))
    if flags not in _CACHE:
        _CACHE[flags] = _build(flags)
    nc = _CACHE[flags]
    use_qb, use_kb, use_vb, use_fb = flags

    def prep_w(w):  # [Cout, Cin, 3] -> [3, Cin, Cout]
        return np.ascontiguousarray(w.transpose(2, 1, 0)).astype(BF16)

    wq_t, wk_t, wv_t, wfc_t = map(prep_w, (wq_w, wk_w, wv_w, fc_w))

    in_maps = []
    for core in range(NCORES):
        b, half = core // 2, core % 2
        qlo = half * HALF
        qpad = np.zeros((C, L + 2), np.float32)
        qpad[:, 1:L + 1] = q[b]
        kpad = np.zeros((C, L + 2), np.float32)
        kpad[:, 1:L + 1] = k[b]
        vpad = np.zeros((C, L + 2), np.float32)
        vpad[:, 1:L + 1] = v[b]
        m = {
            "q_in": qpad[:, qlo:qlo + 1026].astype(BF16),
            "k_in": kpad[:, qlo:qlo + 1026].astype(BF16),
            "v_in": vpad[:, qlo:qlo + 1026].astype(BF16),
            "wq": wq_t, "wk": wk_t, "wv": wv_t, "wfc": wfc_t,
            # hm[0]=mL (left halo valid for half=1), hm[1]=mR (right
            # halo valid for half=0)
            "hm": np.array([[float(half == 1), float(half == 0)]],
                           np.float32),
        }
        if use_qb:
            m["qb"] = wq_b.reshape(1, C)
        if use_kb:
            m["kb"] = wk_b.reshape(1, C)
        if use_vb:
            m["vb"] = np.ascontiguousarray(wv_b.reshape(CIN_CH, 128).T)
        if use_fb:
            m["fb"] = np.ascontiguousarray(fc_b.reshape(CIN_CH, 128).T)
        in_maps.append(m)

    global _LAST_IN_MAPS, _LAST_RESULTS
    _LAST_IN_MAPS = in_maps
    from concourse.bass_utils import run_bass_kernel_spmd
    res = run_bass_kernel_spmd(nc, in_maps, list(range(NCORES))).results
    _LAST_RESULTS = res

    out = np.empty((B, C, L), np.float32)
    for core in range(NCORES):
        b, half = core // 2, core % 2
        out[b][:, half * HALF:(half + 1) * HALF] = res[core]["out"]
    return out
